# revision 1
# baseline (speedup 1.0000x reference)
"""Fused transformer block (QKV proj + attention + FFN + 2x LayerNorm) on 8
Trainium2 NeuronCores.

Sharding: batch (B=2) across two 4-core groups; within a group, tensor
parallel over heads (4 heads / core) for projections+attention, then an
AllToAll switches to row (sequence) sharding for the FFN/LayerNorm tail.

Matmuls run in float32r (full-rate fp32 on the PE array, ~1e-3 rel err);
accumulation is fp32 in PSUM.
"""
import sys

import numpy as np

try:
    import concourse.bass  # noqa: F401
except ImportError:
    sys.path.insert(0, "/opt/trn_rl_repo")

import concourse.bacc as bacc
import concourse.mybir as mybir
import concourse.tile as tile
from concourse import bass_utils
from concourse.masks import make_identity

P = 128
S = 2048          # sequence length (Sq == Sk)
D = 1024          # model dim
H = 16            # total heads
DH = 64           # head dim
NCORES = 8
GROUP = 4         # cores per batch group
JC = D // GROUP   # 256 local projection columns
HL = JC // DH     # 4 local heads
SR = S // GROUP   # 512 output rows per core
DCH = D // P      # 8 d chunks
SCH = S // P      # 16 s chunks
QB = 512          # q block for attention
NQB = S // QB     # 4
F32 = mybir.dt.float32
F32R = mybir.dt.float32r
AF = mybir.ActivationFunctionType
OP = mybir.AluOpType
EPS = 1e-5

_CACHE: dict = {}


def _declare_io(nc):
    t = {}
    t["q"] = nc.dram_tensor("q", [S, D], F32, kind="ExternalInput").ap()
    t["k"] = nc.dram_tensor("k", [S, D], F32, kind="ExternalInput").ap()
    for w in ("wq", "wk", "wv"):
        t[w] = nc.dram_tensor(w, [D, JC], F32, kind="ExternalInput").ap()
    for b in ("bq", "bk", "bv"):
        t[b] = nc.dram_tensor(b, [1, JC], F32, kind="ExternalInput").ap()
    t["wo"] = nc.dram_tensor("wo", [D, D], F32, kind="ExternalInput").ap()
    for b in ("bo", "g0", "b0", "g1", "b1"):
        t[b] = nc.dram_tensor(b, [1, D], F32, kind="ExternalInput").ap()
    t["out"] = nc.dram_tensor("out", [SR, D], F32, kind="ExternalOutput").ap()
    return t


def _transpose_and_project(nc, tc, ctx, pools, x_dram, w_dram, bias_sb, heads_sb,
                           ag_in=None):
    """x [S, D] fp32 DRAM, w [D, JC] -> heads_sb [64, HL, S] f32r (= proj^T,
    head-split, bias added). Optionally DMA the projected slice to ag_in
    ([JC, S] DRAM) for the AllGather."""
    ident = pools["ident_r"]
    xt = pools["xt"].tile([P, DCH, S], F32R, tag="xt")          # x^T, 8 MB
    w_sb = pools["w"].tile([P, DCH, JC], F32R, tag="w")
    nc.gpsimd.dma_start(w_sb[:], w_dram.rearrange("(c p) j -> p c j", p=P))
    # transpose x into xt (PE identity transposes, 128x128 blocks)
    for dc in range(DCH):
        for scg in range(SCH // 4):            # groups of 4 s-blocks / psum bank
            pst = pools["pst"].tile([P, 4 * P], F32R, tag="pst")
            for i in range(4):
                sc = 4 * scg + i
                raw = pools["raw"].tile([P, P], F32R, tag="raw")
                nc.gpsimd.dma_start(
                    raw[:], x_dram[sc * P:(sc + 1) * P, dc * P:(dc + 1) * P])
                nc.tensor.transpose(pst[:, i * P:(i + 1) * P], raw[:], ident)
            nc.vector.tensor_copy(xt[:, dc, 4 * P * scg:4 * P * (scg + 1)], pst[:])
    # project: out^T [JC, S] = w^T x^T, accumulate over d chunks
    for jc2 in range(JC // P):
        for nb in range(S // QB):
            ps = pools["psp"].tile([P, QB], F32, tag="psp")
            for dc in range(DCH):
                nc.tensor.matmul(
                    ps[:], w_sb[:, dc, jc2 * P:(jc2 + 1) * P],
                    xt[:, dc, nb * QB:(nb + 1) * QB],
                    start=(dc == 0), stop=(dc == DCH - 1))
            for hh in range(2):
                h = 2 * jc2 + hh
                nc.vector.tensor_scalar(
                    out=heads_sb[:, h, nb * QB:(nb + 1) * QB],
                    in0=ps[hh * DH:(hh + 1) * DH, :],
                    scalar1=bias_sb[:, h:h + 1], scalar2=None, op0=OP.add)
    if ag_in is not None:
        for h in range(HL):
            nc.sync.dma_start(
                ag_in[h * DH:(h + 1) * DH, :],
                heads_sb[:, h, :].bitcast(F32))


def _emit(nc, tc, ctx, t):
    # SBUF pools are a LIFO stack: open order must be reverse of close order.
    pools = {}
    pools["const"] = ctx.enter_context(tc.tile_pool(name="const", bufs=1))
    heads_cm = tc.tile_pool(name="heads", bufs=1)
    heads = heads_cm.__enter__()
    w_cm = tc.tile_pool(name="w", bufs=2)
    pools["w"] = w_cm.__enter__()
    raw_cm = tc.tile_pool(name="raw", bufs=6)
    pools["raw"] = raw_cm.__enter__()
    xt_cm = tc.tile_pool(name="xt", bufs=1)
    pools["xt"] = xt_cm.__enter__()
    pools["pst"] = ctx.enter_context(tc.tile_pool(name="pst", bufs=2, space="PSUM"))
    psp_cm = tc.tile_pool(name="psp", bufs=3, space="PSUM")
    pools["psp"] = psp_cm.__enter__()
    dram = ctx.enter_context(tc.tile_pool(name="dram", bufs=1, space="DRAM"))
    const = pools["const"]

    # constants
    ident_f = const.tile([P, P], F32)
    make_identity(nc, ident_f)
    ident_r = const.tile([P, P], F32R)
    nc.vector.tensor_copy(ident_r[:], ident_f[:])
    pools["ident_r"] = ident_r
    eps_t = const.tile([P, 1], F32)
    nc.vector.memset(eps_t, EPS)

    # per-partition bias views [64, HL]
    bq_sb = const.tile([DH, HL], F32)
    nc.sync.dma_start(bq_sb[:], t["bq"].rearrange("o (h p) -> (o p) h", p=DH))
    bk_sb = const.tile([DH, HL], F32)
    nc.sync.dma_start(bk_sb[:], t["bk"].rearrange("o (h p) -> (o p) h", p=DH))
    # broadcast params [128, N]
    bvb = const.tile([P, JC], F32)
    nc.gpsimd.dma_start(bvb[:], t["bv"].to_broadcast([P, JC]))
    bob = const.tile([P, D], F32)
    nc.gpsimd.dma_start(bob[:], t["bo"].to_broadcast([P, D]))
    g0b = const.tile([P, D], F32)
    nc.gpsimd.dma_start(g0b[:], t["g0"].to_broadcast([P, D]))
    b0b = const.tile([P, D], F32)
    nc.gpsimd.dma_start(b0b[:], t["b0"].to_broadcast([P, D]))
    g1b = const.tile([P, D], F32)
    nc.gpsimd.dma_start(g1b[:], t["g1"].to_broadcast([P, D]))
    b1b = const.tile([P, D], F32)
    nc.gpsimd.dma_start(b1b[:], t["b1"].to_broadcast([P, D]))

    groups = [list(range(GROUP)), list(range(GROUP, 2 * GROUP))]

    # ---- K path: K^T, Kp^T (own 4 heads), AllGather of Kp^T ----
    k_heads = heads.tile([DH, HL, S], F32R)           # Kp^T per local head
    ag_in = dram.tile([JC, S], F32)
    ag_out = dram.tile([D, S], F32)
    _transpose_and_project(nc, tc, ctx, pools, t["k"], t["wk"], bk_sb, k_heads,
                           ag_in=ag_in)
    nc.gpsimd.collective_compute(
        "AllGather", OP.bypass, ins=[ag_in.opt()], outs=[ag_out.opt()],
        replica_groups=groups)

    # ---- Q path ----
    q_heads = heads.tile([DH, HL, S], F32R)
    _transpose_and_project(nc, tc, ctx, pools, t["q"], t["wq"], bq_sb, q_heads)
    xt_cm.__exit__(None, None, None)          # free x^T (64 KB/partition)

    # ---- Vp natural [S, JC] with fused ones column: [128, SCH, HL, 65] ----
    vp = heads.tile([P, SCH, HL, DH + 1], F32R)
    wv_sb = pools["w"].tile([P, DCH, JC], F32R, tag="w")
    nc.gpsimd.dma_start(wv_sb[:], t["wv"].rearrange("(c p) j -> p c j", p=P))
    for sc in range(SCH):
        psv = pools["psp"].tile([P, JC], F32, tag="psp")
        for dc in range(DCH):
            kpf = pools["raw"].tile([P, P], F32R, tag="raw")
            nc.sync.dma_start(
                kpf[:], ag_out[dc * P:(dc + 1) * P, sc * P:(sc + 1) * P]
                .bitcast(F32R))
            nc.tensor.matmul(psv[:], kpf[:], wv_sb[:, dc, :],
                             start=(dc == 0), stop=(dc == DCH - 1))
        nc.vector.tensor_tensor(
            out=vp[:, sc, :, 0:DH],
            in0=psv.rearrange("p (h d) -> p h d", h=HL),
            in1=bvb.rearrange("p (h d) -> p h d", h=HL), op=OP.add)
    ones_t = const.tile([P, 1], F32)
    nc.vector.memset(ones_t, 1.0)
    nc.vector.tensor_copy(
        vp[:, :, :, DH:DH + 1],
        ones_t[:, None, :].broadcast_to([P, SCH, HL, 1]))
    raw_cm.__exit__(None, None, None)         # free block-load tiles
    w_cm.__exit__(None, None, None)           # free weight slices
    psp_cm.__exit__(None, None, None)         # free projection psum banks

    # ---- attention: per (head, q-block) ----
    att_cm = tc.tile_pool(name="att", bufs=1)
    att = att_cm.__enter__()
    oh = att.tile([DH, HL, S], F32)                   # (Qh + attnV)^T unnormed->final
    spool_cm = tc.tile_pool(name="spool", bufs=4)
    spool = spool_cm.__enter__()
    epool_cm = tc.tile_pool(name="epool", bufs=3)
    epool = epool_cm.__enter__()
    ps_s = ctx.enter_context(tc.tile_pool(name="ps_s", bufs=2, space="PSUM"))
    ps_a_cm = tc.tile_pool(name="ps_a", bufs=2, space="PSUM")
    ps_a = ps_a_cm.__enter__()
    for h in range(HL):
        for qb in range(NQB):
            qsl = slice(qb * QB, (qb + 1) * QB)
            psA = ps_a.tile([DH + 1, QB], F32, tag="psA")
            for g in range(SCH // 2):
                psS = ps_s.tile([P, 2 * QB], F32, tag="psS")
                for i in range(2):
                    kc = 2 * g + i
                    nc.tensor.matmul(
                        psS[:, i * QB:(i + 1) * QB],
                        k_heads[:, h, kc * P:(kc + 1) * P],
                        q_heads[:, h, qsl], start=True, stop=True)
                e_sb = epool.tile([P, 2 * QB], F32R, tag="e")
                nc.scalar.activation(e_sb[:], psS[:], AF.Exp, scale=0.125)
                for i in range(2):
                    kc = 2 * g + i
                    nc.tensor.matmul(
                        psA[:], vp[:, kc, h, :], e_sb[:, i * QB:(i + 1) * QB],
                        start=(kc == 0), stop=(kc == SCH - 1))
            recip = spool.tile([1, QB], F32, tag="recip")
            nc.vector.reciprocal(recip[:], psA[DH:DH + 1, :])
            recipb = spool.tile([DH, QB], F32, tag="recipb")
            nc.gpsimd.partition_broadcast(recipb[:], recip[:], channels=DH)
            nc.vector.tensor_tensor(out=oh[:, h, qsl], in0=psA[0:DH, :],
                                    in1=recipb[:], op=OP.mult)
            nc.vector.tensor_tensor(out=oh[:, h, qsl], in0=oh[:, h, qsl],
                                    in1=q_heads[:, h, qsl], op=OP.add)

    # ---- transpose heads to natural rows, AllToAll to row sharding ----
    a2a_in = dram.tile([S, JC], F32)
    a2a_out = dram.tile([S, JC], F32)
    for sc in range(SCH):
        psT = pools["pst"].tile([P, JC], F32, tag="pst")
        for h in range(HL):
            nc.tensor.transpose(psT[:, h * DH:(h + 1) * DH],
                                oh[:, h, sc * P:(sc + 1) * P],
                                ident_f[0:DH, 0:DH])
        stg = spool.tile([P, JC], F32, tag="stg")
        nc.vector.tensor_copy(stg[:], psT[:])
        nc.sync.dma_start(a2a_in[sc * P:(sc + 1) * P, :], stg[:])
    nc.gpsimd.collective_compute(
        "AllToAll", OP.bypass, ins=[a2a_in.opt()], outs=[a2a_out.opt()],
        replica_groups=[list(range(NCORES))])
    ps_a_cm.__exit__(None, None, None)
    epool_cm.__exit__(None, None, None)
    spool_cm.__exit__(None, None, None)
    att_cm.__exit__(None, None, None)         # free oh (32 KB)
    heads_cm.__exit__(None, None, None)       # free k/q heads + vp (80 KB)

    # ---- stage 2: rows [SR, D] : LN0 -> FFN(Wo)+gelu+residual -> LN1 ----
    s2 = ctx.enter_context(tc.tile_pool(name="s2", bufs=1))
    ln_tmp = ctx.enter_context(tc.tile_pool(name="ln_tmp", bufs=4))
    NS2 = SR // P                                     # 4 row chunks
    # 8-rank AllToAll: shard p of a2a_out = rows [256c:256c+256) x cols
    # [256(p%4):...) of batch p//4. Chunks 0,1 -> batch 0; chunks 2,3 -> b 1.
    o_sb = s2.tile([P, NS2, D], F32)
    for sc2 in range(NS2):
        bb, rr = divmod(sc2, 2)
        for j in range(GROUP):
            pr = bb * GROUP + j
            base = pr * (S // NCORES) + rr * P
            nc.sync.dma_start(
                o_sb[:, sc2, j * JC:(j + 1) * JC],
                a2a_out[base:base + P, :])

    def layernorm(src_ap, dst_ap, gb, bb, sc2):
        """src [128, D] -> dst [128, D] layernorm with broadcast gamma/beta."""
        red = ln_tmp.tile([P, 1], F32, tag="red")
        nc.vector.tensor_reduce(red[:], src_ap, mybir.AxisListType.X, OP.add)
        negmean = ln_tmp.tile([P, 1], F32, tag="negmean")
        nc.vector.tensor_scalar_mul(negmean[:], red[:], -1.0 / D)
        sq = ln_tmp.tile([P, D], F32, tag="sq")
        sumsq = ln_tmp.tile([P, 1], F32, tag="sumsq")
        nc.scalar.activation(sq[:], src_ap, AF.Square, bias=negmean[:],
                             scale=1.0, accum_out=sumsq[:])
        std = ln_tmp.tile([P, 1], F32, tag="std")
        nc.scalar.activation(std[:], sumsq[:], AF.Sqrt, bias=eps_t[:],
                             scale=1.0 / D)
        rstd = ln_tmp.tile([P, 1], F32, tag="rstd")
        nc.vector.reciprocal(rstd[:], std[:])
        nc.vector.tensor_scalar(out=dst_ap, in0=src_ap, scalar1=negmean[:],
                                scalar2=rstd[:], op0=OP.add, op1=OP.mult)
        nc.vector.tensor_tensor(out=dst_ap, in0=dst_ap, in1=gb[:], op=OP.mult)
        nc.vector.tensor_tensor(out=dst_ap, in0=dst_ap, in1=bb[:], op=OP.add)

    ln0 = s2.tile([P, NS2, D], F32R)
    for sc2 in range(NS2):
        layernorm(o_sb[:, sc2, :], ln0[:, sc2, :], g0b, b0b, sc2)

    # transpose ln0 -> [128, DCH, SR] for the Wo contraction
    ln0t = s2.tile([P, DCH, SR], F32R)
    for dc in range(DCH):
        psL = pools["pst"].tile([P, SR], F32R, tag="pst")
        for sc2 in range(NS2):
            nc.tensor.transpose(psL[:, sc2 * P:(sc2 + 1) * P],
                                ln0[:, sc2, dc * P:(dc + 1) * P], ident_r)
        nc.vector.tensor_copy(ln0t[:, dc, :], psL[:])

    wo_sb = s2.tile([P, DCH, D], F32R)
    nc.gpsimd.dma_start(wo_sb[:], t["wo"].rearrange("(c p) j -> p c j", p=P))
    o2 = s2.tile([P, NS2, D], F32)
    for sc2 in range(NS2):
        psF = ps_s.tile([P, D], F32, tag="psS")
        for dc in range(DCH):
            for nb in range(2):
                nc.tensor.matmul(
                    psF[:, nb * QB:(nb + 1) * QB],
                    ln0t[:, dc, sc2 * P:(sc2 + 1) * P],
                    wo_sb[:, dc, nb * QB:(nb + 1) * QB],
                    start=(dc == 0), stop=(dc == DCH - 1))
        fb = ln_tmp.tile([P, D], F32, tag="fb")
        nc.vector.tensor_tensor(out=fb[:], in0=psF[:], in1=bob[:], op=OP.add)
        gel = ln_tmp.tile([P, D], F32, tag="gel")
        nc.scalar.activation(gel[:], fb[:], AF.Gelu)
        nc.vector.tensor_tensor(out=o2[:, sc2, :], in0=ln0[:, sc2, :],
                                in1=gel[:], op=OP.add)

    for sc2 in range(NS2):
        fin = ln_tmp.tile([P, D], F32, tag="fin")
        layernorm(o2[:, sc2, :], fin[:], g1b, b1b, sc2)
        nc.sync.dma_start(t["out"][sc2 * P:(sc2 + 1) * P, :], fin[:])


def build():
    if "nc" in _CACHE:
        return _CACHE["nc"]
    from contextlib import ExitStack
    nc = bacc.Bacc("TRN2", target_bir_lowering=False, debug=False,
                   num_devices=NCORES)
    t = _declare_io(nc)
    with tile.TileContext(nc) as tc:
        with ExitStack() as ctx:
            _emit(nc, tc, ctx, t)
    nc.compile()
    _CACHE["nc"] = nc
    return nc


def make_in_maps(Q, K, Wq, bq, Wk, bk, Wv, bv, Wo, bo, g0, b0, g1, b1):
    in_maps = []
    for c in range(NCORES):
        b, g = divmod(c, GROUP)
        jsl = slice(g * JC, (g + 1) * JC)
        ac = np.ascontiguousarray
        in_maps.append({
            "q": ac(Q[b]), "k": ac(K[b]),
            "wq": ac(Wq[:, jsl]), "wk": ac(Wk[:, jsl]), "wv": ac(Wv[:, jsl]),
            "bq": ac(bq[jsl].reshape(1, JC)), "bk": ac(bk[jsl].reshape(1, JC)),
            "bv": ac(bv[jsl].reshape(1, JC)),
            "wo": ac(Wo), "bo": ac(bo.reshape(1, D)),
            "g0": ac(g0.reshape(1, D)), "b0": ac(b0.reshape(1, D)),
            "g1": ac(g1.reshape(1, D)), "b1": ac(b1.reshape(1, D)),
        })
    return in_maps


def run(in_maps, trace=False, **kwargs):
    nc = build()
    return bass_utils.run_bass_kernel_spmd(
        nc, in_maps, core_ids=list(range(NCORES)), trace=trace, **kwargs)


def kernel(**inputs):
    inputs = {k: np.asarray(v, dtype=np.float32) for k, v in inputs.items()}
    in_maps = make_in_maps(
        inputs["Q"], inputs["K"], inputs["Wq"], inputs["bq"], inputs["Wk"],
        inputs["bk"], inputs["Wv"], inputs["bv"], inputs["Wo"], inputs["bo"],
        inputs["g0"], inputs["b0"], inputs["g1"], inputs["b1"])
    res = run(in_maps, trace=False)
    B = 2
    RS = S // NCORES  # 256 rows of each batch per core
    out = np.empty((B, S, D), dtype=np.float32)
    for c in range(NCORES):
        r = res.results[c]["out"]  # [512, D]: rows 0-255 -> b0, 256-511 -> b1
        out[0, c * RS:(c + 1) * RS, :] = r[:RS]
        out[1, c * RS:(c + 1) * RS, :] = r[RS:]
    return out


if __name__ == "__main__":
    rng = np.random.default_rng(0)
    ins = {n: rng.standard_normal(s).astype(np.float32) * (0.03125 if n.startswith("w") else 1.0)
           for n, s in [("Q", (2, S, D)), ("K", (2, S, D)), ("Wq", (D, D)),
                        ("Wk", (D, D)), ("Wv", (D, D)), ("Wo", (D, D))]}
    for n in ("bq", "bk", "bv", "bo", "b0", "b1"):
        ins[n] = np.zeros(D, np.float32)
    for n in ("g0", "g1"):
        ins[n] = np.ones(D, np.float32)
    out = kernel(**ins)
    print("ran ok", out.shape, out.dtype)



# revision 7
# speedup vs baseline: 1.1600x; 1.1600x over previous
"""Fused transformer block (QKV proj + attention + FFN + 2x LayerNorm) on 8
Trainium2 NeuronCores.

Sharding: batch (B=2) across two 4-core groups; within a group, tensor
parallel over heads (4 heads / core) for projections+attention, then a
2-chunk AllToAll switches to row sharding for the FFN/LayerNorm tail.

v2: fp16/bf16 matmul operands (1 cyc/row PE streams), host-folded
Wkv = Wk@Wv kills the AllGather, chunked fp16 AllToAll overlaps the tail
with the second half of attention, softmax exp is the only large Act-engine
consumer, projections' bias-adds ride the idle Act/DVE engines.
"""
import sys

import numpy as np

try:
    import concourse.bass  # noqa: F401
except ImportError:
    sys.path.insert(0, "/opt/trn_rl_repo")

import concourse.bacc as bacc
import concourse.mybir as mybir
import concourse.tile as tile
from concourse import bass_utils
from concourse.masks import make_identity

P = 128
S = 2048          # sequence length (Sq == Sk)
D = 1024          # model dim
H = 16            # total heads
DH = 64           # head dim
NCORES = 8
GROUP = 4         # cores per batch group
JC = D // GROUP   # 256 local projection columns
HL = JC // DH     # 4 local heads
DCH = D // P      # 8 d chunks
SCH = S // P      # 16 s chunks
QBP = 1024        # q block for attention units (2 per head)
F32 = mybir.dt.float32
F16 = mybir.dt.float16
BF16 = mybir.dt.bfloat16
AF = mybir.ActivationFunctionType
OP = mybir.AluOpType
EPS = 1e-5

_CACHE: dict = {}


def _declare_io(nc):
    t = {}
    t["q"] = nc.dram_tensor("q", [S, D], F16, kind="ExternalInput").ap()
    t["k"] = nc.dram_tensor("k", [S, D], F16, kind="ExternalInput").ap()
    for w in ("wq", "wk", "wkv"):
        t[w] = nc.dram_tensor(w, [D, JC], F16, kind="ExternalInput").ap()
    for b in ("bqp", "bkp"):
        t[b] = nc.dram_tensor(b, [P, 2], F32, kind="ExternalInput").ap()
    t["bvv"] = nc.dram_tensor("bvv", [1, JC], F32, kind="ExternalInput").ap()
    t["wo"] = nc.dram_tensor("wo", [D, D], F16, kind="ExternalInput").ap()
    for b in ("bo", "g0", "b0", "g1", "b1"):
        t[b] = nc.dram_tensor(b, [1, D], F32, kind="ExternalInput").ap()
    t["out"] = nc.dram_tensor("out", [4 * P, D], F32, kind="ExternalOutput").ap()
    return t


def _x_path(nc, pools, x_dram, w_sb, bias_sb, heads_sb, xt, wkv_sb=None,
            vp=None, bvb=None):
    """Load x [S, D] f16, transpose to xt [128, DCH, S], project to
    heads_sb [64, HL, S] (= proj^T, head-split, bias added). If wkv_sb is
    given, also compute vp (natural-layout V projection via the folded
    K->V weight) with a fused ones column."""
    ident = pools["ident"]
    for sc in range(SCH):
        xr = pools["xraw"].tile([P, D], F16, tag="xr")
        nc.sync.dma_start(xr[:], x_dram[sc * P:(sc + 1) * P, :])
        for dcg in range(2):
            pstt = pools["pst"].tile([P, 4 * P], F16, tag="pst")
            for i in range(4):
                dc = 4 * dcg + i
                nc.tensor.transpose(pstt[:, i * P:(i + 1) * P],
                                    xr[:, dc * P:(dc + 1) * P], ident)
            dst = xt[:, 4 * dcg:4 * dcg + 4, sc * P:(sc + 1) * P]
            src = pstt.rearrange("p (c q) -> p c q", c=4)
            if (sc + dcg) % 2 == 0:
                nc.vector.tensor_copy(dst, src)
            else:
                nc.scalar.copy(dst, src)
    # proj^T [JC, S]: accumulate over d chunks; bias-add splits head pairs
    for jc2 in range(2):
        for sh in range(2):
            ps = pools["ps2"].tile([P, QBP], F32, tag="ps2")
            for dc in range(DCH):
                for nb in range(2):
                    nc.tensor.matmul(
                        ps[:, nb * 512:(nb + 1) * 512],
                        w_sb[:, dc, jc2 * P:(jc2 + 1) * P],
                        xt[:, dc, sh * QBP + nb * 512:sh * QBP + (nb + 1) * 512],
                        start=(dc == 0), stop=(dc == DCH - 1))
            ssl = slice(sh * QBP, (sh + 1) * QBP)
            nc.scalar.activation(
                heads_sb[:, 2 * jc2, ssl], ps[0:DH, :], AF.Identity,
                bias=bias_sb[0:DH, jc2:jc2 + 1], scale=1.0)
            nc.vector.tensor_scalar(
                out=heads_sb[:, 2 * jc2 + 1, ssl], in0=ps[DH:P, :],
                scalar1=bias_sb[DH:P, jc2:jc2 + 1], scalar2=None, op0=OP.add)
    if wkv_sb is not None:
        # Vp natural [S, JC] = K @ (Wk Wv), bias bkv, + ones column
        for sc in range(SCH):
            psv = pools["ps2"].tile([P, QBP], F32, tag="ps2")
            for dc in range(DCH):
                nc.tensor.matmul(
                    psv[:, 0:JC], xt[:, dc, sc * P:(sc + 1) * P],
                    wkv_sb[:, dc, :], start=(dc == 0), stop=(dc == DCH - 1))
            nc.vector.tensor_tensor(
                out=vp[:, sc, :, 0:DH],
                in0=psv[:, 0:JC].rearrange("p (h d) -> p h d", h=HL),
                in1=bvb.rearrange("p (h d) -> p h d", h=HL), op=OP.add)
        nc.gpsimd.memset(vp[:, :, :, DH:DH + 1], 1.0)


def _emit(nc, tc, ctx, t):
    pools = {}
    pools["const"] = ctx.enter_context(tc.tile_pool(name="const", bufs=1))
    persist_cm = tc.tile_pool(name="persist", bufs=1)
    persist = persist_cm.__enter__()
    dram = ctx.enter_context(tc.tile_pool(name="dram", bufs=1, space="DRAM"))
    pools["pst"] = ctx.enter_context(tc.tile_pool(name="pst", bufs=2, space="PSUM"))
    pools["ps2"] = ctx.enter_context(tc.tile_pool(name="ps2", bufs=2, space="PSUM"))
    ps_a = ctx.enter_context(tc.tile_pool(name="ps_a", bufs=1, space="PSUM"))
    const = pools["const"]

    # constants
    ident = const.tile([P, P], F16)
    make_identity(nc, ident)
    pools["ident"] = ident
    eps_t = const.tile([P, 1], F32)
    nc.vector.memset(eps_t, EPS)
    bqp = const.tile([P, 2], F32)
    nc.gpsimd.dma_start(bqp[:], t["bqp"])
    bkp = const.tile([P, 2], F32)
    nc.gpsimd.dma_start(bkp[:], t["bkp"])
    bvb = const.tile([P, JC], F32)
    nc.gpsimd.dma_start(bvb[:], t["bvv"].to_broadcast([P, JC]))
    bob = const.tile([P, D], F32)
    nc.gpsimd.dma_start(bob[:], t["bo"].to_broadcast([P, D]))
    g0b = const.tile([P, D], F32)
    nc.gpsimd.dma_start(g0b[:], t["g0"].to_broadcast([P, D]))
    b0b = const.tile([P, D], F32)
    nc.gpsimd.dma_start(b0b[:], t["b0"].to_broadcast([P, D]))
    g1b = const.tile([P, D], F32)
    nc.gpsimd.dma_start(g1b[:], t["g1"].to_broadcast([P, D]))
    b1b = const.tile([P, D], F32)
    nc.gpsimd.dma_start(b1b[:], t["b1"].to_broadcast([P, D]))

    # persistent tiles
    k_heads = persist.tile([DH, HL, S], F16)
    q_heads = persist.tile([DH, HL, S], F16)
    vp = persist.tile([P, SCH, HL, DH + 1], BF16)
    oh = persist.tile([DH, HL, S], F16)
    wo_sb = persist.tile([P, DCH, D], F16)
    nc.gpsimd.dma_start(wo_sb[:], t["wo"].rearrange("(c p) j -> p c j", p=P))

    a2a_in = [dram.tile([QBP, JC], F16, name=f"a2a_in{i}") for i in range(2)]
    a2a_out = [dram.tile([QBP, JC], F16, name=f"a2a_out{i}") for i in range(2)]

    # ---- phase 1: K path (with folded V), then Q path ----
    w_cm = tc.tile_pool(name="w", bufs=1)
    wpool = w_cm.__enter__()
    wk_sb = wpool.tile([P, DCH, JC], F16)
    nc.gpsimd.dma_start(wk_sb[:], t["wk"].rearrange("(c p) j -> p c j", p=P))
    wkv_sb = wpool.tile([P, DCH, JC], F16)
    nc.gpsimd.dma_start(wkv_sb[:], t["wkv"].rearrange("(c p) j -> p c j", p=P))
    wq_sb = wpool.tile([P, DCH, JC], F16)
    nc.gpsimd.dma_start(wq_sb[:], t["wq"].rearrange("(c p) j -> p c j", p=P))

    xt_cm = tc.tile_pool(name="xt", bufs=1)
    xtp = xt_cm.__enter__()
    xtk = xtp.tile([P, DCH, S], F16)
    xraw_cm = tc.tile_pool(name="xraw", bufs=3)
    pools["xraw"] = xraw_cm.__enter__()
    _x_path(nc, pools, t["k"], wk_sb, bkp, k_heads, xtk,
            wkv_sb=wkv_sb, vp=vp, bvb=bvb)
    xraw_cm.__exit__(None, None, None)
    xt_cm.__exit__(None, None, None)

    xt_cm2 = tc.tile_pool(name="xt2", bufs=1)
    xtp2 = xt_cm2.__enter__()
    xtq = xtp2.tile([P, DCH, S], F16)
    xraw_cm2 = tc.tile_pool(name="xraw2", bufs=3)
    pools["xraw"] = xraw_cm2.__enter__()
    _x_path(nc, pools, t["q"], wq_sb, bqp, q_heads, xtq)
    xraw_cm2.__exit__(None, None, None)
    xt_cm2.__exit__(None, None, None)
    w_cm.__exit__(None, None, None)

    # ---- attention + chunked AllToAll + tail ----
    att_cm = tc.tile_pool(name="att", bufs=2)
    att = att_cm.__enter__()
    epool_cm = tc.tile_pool(name="epool", bufs=3)
    epool = epool_cm.__enter__()
    tail_cm = tc.tile_pool(name="tail", bufs=2)
    tailp = tail_cm.__enter__()

    def layernorm(src_ap, dst_ap, gb, bb):
        """src [128, D] -> dst [128, D] layernorm; gamma/beta on gpsimd."""
        red = tailp.tile([P, 1], F32, tag="red")
        nc.vector.tensor_reduce(red[:], src_ap, mybir.AxisListType.X, OP.add)
        negmean = tailp.tile([P, 1], F32, tag="negmean")
        nc.vector.tensor_scalar_mul(negmean[:], red[:], -1.0 / D)
        sq = tailp.tile([P, D], F32, tag="sq")
        sumsq = tailp.tile([P, 1], F32, tag="sumsq")
        nc.scalar.activation(sq[:], src_ap, AF.Square, bias=negmean[:],
                             scale=1.0, accum_out=sumsq[:])
        std = tailp.tile([P, 1], F32, tag="std")
        nc.scalar.activation(std[:], sumsq[:], AF.Sqrt, bias=eps_t[:],
                             scale=1.0 / D)
        rstd = tailp.tile([P, 1], F32, tag="rstd")
        nc.vector.reciprocal(rstd[:], std[:])
        nc.vector.tensor_scalar(out=dst_ap, in0=src_ap, scalar1=negmean[:],
                                scalar2=rstd[:], op0=OP.add, op1=OP.mult)
        nc.gpsimd.tensor_tensor(out=dst_ap, in0=dst_ap, in1=gb[:], op=OP.mult)
        nc.gpsimd.tensor_tensor(out=dst_ap, in0=dst_ap, in1=bb[:], op=OP.add)

    for qbp in range(2):
        qsl = slice(qbp * QBP, (qbp + 1) * QBP)
        for h in range(HL):
            psA = ps_a.tile([DH + 1, QBP], F32, tag="psA")
            for kc in range(SCH):
                pss = pools["ps2"].tile([P, QBP], F32, tag="ps2")
                for nb in range(2):
                    nc.tensor.matmul(
                        pss[:, nb * 512:(nb + 1) * 512],
                        k_heads[:, h, kc * P:(kc + 1) * P],
                        q_heads[:, h, qbp * QBP + nb * 512:qbp * QBP + (nb + 1) * 512],
                        start=True, stop=True)
                e_sb = epool.tile([P, QBP], BF16, tag="e")
                nc.scalar.activation(e_sb[:], pss[:], AF.Exp, scale=0.125)
                for nb in range(2):
                    nc.tensor.matmul(
                        psA[:, nb * 512:(nb + 1) * 512], vp[:, kc, h, :],
                        e_sb[:, nb * 512:(nb + 1) * 512],
                        start=(kc == 0), stop=(kc == SCH - 1))
            recip = att.tile([1, QBP], F32, tag="recip")
            nc.vector.reciprocal(recip[:], psA[DH:DH + 1, :])
            recipb = att.tile([DH, QBP], F32, tag="recipb")
            nc.gpsimd.partition_broadcast(recipb[:], recip[:], channels=DH)
            nc.vector.tensor_tensor(out=oh[:, h, qsl], in0=psA[0:DH, :],
                                    in1=recipb[:], op=OP.mult)
            nc.vector.tensor_tensor(out=oh[:, h, qsl], in0=oh[:, h, qsl],
                                    in1=q_heads[:, h, qsl], op=OP.add)
        # transpose this q-chunk to natural rows, fire chunk AllToAll
        for scq in range(QBP // P):
            pstt = pools["pst"].tile([P, 4 * P], F16, tag="pst")
            for h in range(HL):
                nc.tensor.transpose(
                    pstt[:, h * DH:(h + 1) * DH],
                    oh[:, h, qbp * QBP + scq * P:qbp * QBP + (scq + 1) * P],
                    ident[0:DH, 0:DH])
            stg = att.tile([P, JC], F16, tag="stg")
            nc.vector.tensor_copy(stg[:], pstt[:, 0:JC])
            nc.sync.dma_start(a2a_in[qbp][scq * P:(scq + 1) * P, :], stg[:])
        nc.gpsimd.collective_compute(
            "AllToAll", OP.bypass, ins=[a2a_in[qbp].opt()],
            outs=[a2a_out[qbp].opt()], replica_groups=[list(range(NCORES))])
        # tail for this chunk: rows [qbp*1024 + rank*128] of both batches
        for b2 in range(2):
            osb = tailp.tile([P, D], F16, tag="osb")
            for j in range(GROUP):
                nc.sync.dma_start(
                    osb[:, j * JC:(j + 1) * JC],
                    a2a_out[qbp][(GROUP * b2 + j) * P:(GROUP * b2 + j + 1) * P, :])
            ln0 = tailp.tile([P, D], F16, tag="ln0")
            layernorm(osb[:], ln0[:], g0b, b0b)
            ln0t = tailp.tile([P, DCH, P], F16, tag="ln0t")
            for dcg in range(2):
                pstt = pools["pst"].tile([P, 4 * P], F16, tag="pst")
                for i in range(4):
                    dc = 4 * dcg + i
                    nc.tensor.transpose(pstt[:, i * P:(i + 1) * P],
                                        ln0[:, dc * P:(dc + 1) * P], ident)
                nc.vector.tensor_copy(
                    ln0t[:, 4 * dcg:4 * dcg + 4, :],
                    pstt.rearrange("p (c q) -> p c q", c=4))
            pso = pools["ps2"].tile([P, QBP], F32, tag="ps2")
            for dc in range(DCH):
                for nb in range(2):
                    nc.tensor.matmul(
                        pso[:, nb * 512:(nb + 1) * 512], ln0t[:, dc, :],
                        wo_sb[:, dc, nb * 512:(nb + 1) * 512],
                        start=(dc == 0), stop=(dc == DCH - 1))
            fb = tailp.tile([P, D], F32, tag="fb")
            nc.vector.tensor_tensor(out=fb[:], in0=pso[:], in1=bob[:], op=OP.add)
            gel = tailp.tile([P, D], F32, tag="gel")
            nc.scalar.activation(gel[:], fb[:], AF.Gelu)
            o2 = tailp.tile([P, D], F32, tag="o2")
            nc.vector.tensor_tensor(out=o2[:], in0=ln0[:], in1=gel[:], op=OP.add)
            fin = tailp.tile([P, D], F32, tag="fin")
            layernorm(o2[:], fin[:], g1b, b1b)
            nc.sync.dma_start(
                t["out"][(2 * qbp + b2) * P:(2 * qbp + b2 + 1) * P, :], fin[:])

    tail_cm.__exit__(None, None, None)
    epool_cm.__exit__(None, None, None)
    att_cm.__exit__(None, None, None)
    persist_cm.__exit__(None, None, None)


def build():
    if "nc" in _CACHE:
        return _CACHE["nc"]
    from contextlib import ExitStack
    nc = bacc.Bacc("TRN2", target_bir_lowering=False, debug=False,
                   num_devices=NCORES)
    t = _declare_io(nc)
    with tile.TileContext(nc) as tc:
        with ExitStack() as ctx:
            _emit(nc, tc, ctx, t)
    nc.compile()
    _CACHE["nc"] = nc
    return nc


def make_in_maps(Q, K, Wq, bq, Wk, bk, Wv, bv, Wo, bo, g0, b0, g1, b1):
    f16 = np.float16
    f32 = np.float32
    Wkv = (Wk.astype(f32) @ Wv.astype(f32))
    bkv = (bk.astype(f32) @ Wv.astype(f32) + bv.astype(f32))
    Qh = [np.ascontiguousarray(Q[b].astype(f16)) for b in range(2)]
    Kh = [np.ascontiguousarray(K[b].astype(f16)) for b in range(2)]
    Wo16 = np.ascontiguousarray(Wo.astype(f16))
    in_maps = []
    for c in range(NCORES):
        b, g = divmod(c, GROUP)
        jsl = slice(g * JC, (g + 1) * JC)
        ac = np.ascontiguousarray
        in_maps.append({
            "q": Qh[b], "k": Kh[b],
            "wq": ac(Wq[:, jsl].astype(f16)),
            "wk": ac(Wk[:, jsl].astype(f16)),
            "wkv": ac(Wkv[:, jsl].astype(f16)),
            "bqp": ac(bq[jsl].astype(f32).reshape(2, P).T),
            "bkp": ac(bk[jsl].astype(f32).reshape(2, P).T),
            "bvv": ac(bkv[jsl].reshape(1, JC)),
            "wo": Wo16, "bo": ac(bo.astype(f32).reshape(1, D)),
            "g0": ac(g0.astype(f32).reshape(1, D)),
            "b0": ac(b0.astype(f32).reshape(1, D)),
            "g1": ac(g1.astype(f32).reshape(1, D)),
            "b1": ac(b1.astype(f32).reshape(1, D)),
        })
    return in_maps


def run(in_maps, trace=False, **kwargs):
    nc = build()
    return bass_utils.run_bass_kernel_spmd(
        nc, in_maps, core_ids=list(range(NCORES)), trace=trace, **kwargs)


def kernel(**inputs):
    inputs = {k: np.asarray(v) for k, v in inputs.items()}
    in_maps = make_in_maps(
        inputs["Q"], inputs["K"], inputs["Wq"], inputs["bq"], inputs["Wk"],
        inputs["bk"], inputs["Wv"], inputs["bv"], inputs["Wo"], inputs["bo"],
        inputs["g0"], inputs["b0"], inputs["g1"], inputs["b1"])
    res = run(in_maps, trace=False)
    out = np.empty((2, S, D), dtype=np.float32)
    for c in range(NCORES):
        r = res.results[c]["out"]  # [512, D] blocks: (qbp0,b0),(qbp0,b1),(qbp1,b0),(qbp1,b1)
        for qbp in range(2):
            for b in range(2):
                out[b, qbp * QBP + c * P:qbp * QBP + (c + 1) * P, :] = \
                    r[(2 * qbp + b) * P:(2 * qbp + b + 1) * P]
    return out


if __name__ == "__main__":
    rng = np.random.default_rng(0)
    ins = {n: rng.standard_normal(s).astype(np.float32) * (0.03125 if n.startswith("W") else 1.0)
           for n, s in [("Q", (2, S, D)), ("K", (2, S, D)), ("Wq", (D, D)),
                        ("Wk", (D, D)), ("Wv", (D, D)), ("Wo", (D, D))]}
    for n in ("bq", "bk", "bv", "bo", "b0", "b1"):
        ins[n] = np.zeros(D, np.float32)
    for n in ("g0", "g1"):
        ins[n] = np.ones(D, np.float32)
    out = kernel(**ins)
    print("ran ok", out.shape, out.dtype)


# revision 16
# speedup vs baseline: 1.6012x; 1.3803x over previous
"""Fused transformer block (QKV proj + attention + FFN + 2x LayerNorm) on 8
Trainium2 NeuronCores.

Sharding: batch (B=2) across two 4-core groups; within a group, tensor
parallel over heads (4 heads / core) for projections+attention, then a
2-chunk AllToAll switches to row sharding for the FFN/LayerNorm tail.

v3: fp8e4m3 DoubleRow matmuls (0.5 cyc/row) for scores, attnV, and the
K-side projections (K/V feed softmax paths where quantization noise
averages out; the 1/sqrt(dh) scale keeps score errors ~0.5%). Q path and
FFN stay fp16 for the residual/output precision. Host-folded Wkv = Wk@Wv
kills the AllGather; a 2-chunk fp16 AllToAll overlaps the tail with the
second half of attention; softmax exp owns the Act engine.
"""
import sys

import numpy as np

try:
    import concourse.bass  # noqa: F401
except ImportError:
    sys.path.insert(0, "/opt/trn_rl_repo")

import concourse.bacc as bacc
import concourse.mybir as mybir
import concourse.tile as tile
from concourse import bass_utils
from concourse.masks import make_identity

P = 128
S = 2048          # sequence length (Sq == Sk)
D = 1024          # model dim
H = 16            # total heads
DH = 64           # head dim
NCORES = 8
GROUP = 4         # cores per batch group
JC = D // GROUP   # 256 local projection columns
HL = JC // DH     # 4 local heads
DCH = D // P      # 8 d chunks
SCH = S // P      # 16 s chunks
QBP = 1024        # q rows per attention unit (2 units per head)
F32 = mybir.dt.float32
F16 = mybir.dt.float16
BF16 = mybir.dt.bfloat16
F8 = mybir.dt.float8e4
F8E5 = mybir.dt.float8e5
AF = mybir.ActivationFunctionType
OP = mybir.AluOpType
DR = mybir.MatmulPerfMode.DoubleRow
EPS = 1e-5

_CACHE: dict = {}


def _declare_io(nc):
    t = {}
    t["q"] = nc.dram_tensor("q", [S, D], F16, kind="ExternalInput").ap()
    t["k"] = nc.dram_tensor("k", [S, D], F16, kind="ExternalInput").ap()
    t["wq"] = nc.dram_tensor("wq", [D, JC], F16, kind="ExternalInput").ap()
    t["wk"] = nc.dram_tensor("wk", [D, JC], F8, kind="ExternalInput").ap()
    t["wkv"] = nc.dram_tensor("wkv", [D, JC], F8, kind="ExternalInput").ap()
    for b in ("bqp", "bkp"):
        t[b] = nc.dram_tensor(b, [P, 2], F32, kind="ExternalInput").ap()
    t["bvv"] = nc.dram_tensor("bvv", [1, JC], F32, kind="ExternalInput").ap()
    t["wo"] = nc.dram_tensor("wo", [D, D], F16, kind="ExternalInput").ap()
    for b in ("bo", "g0", "b0", "g1", "b1"):
        t[b] = nc.dram_tensor(b, [1, D], F32, kind="ExternalInput").ap()
    t["out"] = nc.dram_tensor("out", [4 * P, D], F32, kind="ExternalOutput").ap()
    return t


def _transpose_in(nc, pools, x_dram, xt, out_dt):
    """x [S, D] f16 DRAM -> xt [128, DCH, S] (dtype out_dt) via PE transposes."""
    ident = pools["ident"]
    for sc in range(SCH):
        xr = pools["xraw"].tile([P, D], F16, tag="xr")
        nc.sync.dma_start(xr[:], x_dram[sc * P:(sc + 1) * P, :])
        for dcg in range(2):
            pstt = pools["pst"].tile([P, 4 * P], F16, tag="pst")
            for i in range(4):
                dc = 4 * dcg + i
                nc.tensor.transpose(pstt[:, i * P:(i + 1) * P],
                                    xr[:, dc * P:(dc + 1) * P], ident)
            dst = xt[:, 4 * dcg:4 * dcg + 4, sc * P:(sc + 1) * P]
            src = pstt.rearrange("p (c q) -> p c q", c=4)
            if (sc + dcg) % 2 == 0:
                nc.vector.tensor_copy(dst, src)
            else:
                nc.scalar.copy(dst, src)


def _emit(nc, tc, ctx, t):
    pools = {}
    pools["const"] = ctx.enter_context(tc.tile_pool(name="const", bufs=1))
    persist_cm = tc.tile_pool(name="persist", bufs=1)
    persist = persist_cm.__enter__()
    dram = ctx.enter_context(tc.tile_pool(name="dram", bufs=1, space="DRAM"))
    pools["pst"] = ctx.enter_context(tc.tile_pool(name="pst", bufs=2, space="PSUM"))
    pools["ps2"] = ctx.enter_context(tc.tile_pool(name="ps2", bufs=2, space="PSUM"))
    ps_a = ctx.enter_context(tc.tile_pool(name="ps_a", bufs=1, space="PSUM"))
    const = pools["const"]

    # constants
    ident = const.tile([P, P], F16)
    make_identity(nc, ident)
    pools["ident"] = ident
    eps_t = const.tile([P, 1], F32)
    nc.vector.memset(eps_t, EPS)
    neg3 = const.tile([P, 1], F32)
    nc.vector.memset(neg3, -3.0)
    pools["neg3"] = neg3
    bqp = const.tile([P, 2], F32)
    nc.gpsimd.dma_start(bqp[:], t["bqp"])
    bkp = const.tile([P, 2], F32)
    nc.gpsimd.dma_start(bkp[:], t["bkp"])
    bvb = const.tile([P, JC], F32)
    nc.gpsimd.dma_start(bvb[:], t["bvv"].to_broadcast([P, JC]))
    bob = const.tile([P, D], F32)
    nc.gpsimd.dma_start(bob[:], t["bo"].to_broadcast([P, D]))
    g0b = const.tile([P, D], F32)
    nc.gpsimd.dma_start(g0b[:], t["g0"].to_broadcast([P, D]))
    b0b = const.tile([P, D], F32)
    nc.gpsimd.dma_start(b0b[:], t["b0"].to_broadcast([P, D]))
    g1b = const.tile([P, D], F32)
    nc.gpsimd.dma_start(g1b[:], t["g1"].to_broadcast([P, D]))
    b1b = const.tile([P, D], F32)
    nc.gpsimd.dma_start(b1b[:], t["b1"].to_broadcast([P, D]))

    # persistent tiles
    k_heads = persist.tile([DH, HL, S], F16)
    q_heads = persist.tile([DH, HL, S], F16)
    # vp8: [k%128, kc//2, kc%2 slab, head, dh+ones+zeros] fp8, M=128 for
    # DoubleRow attnV (stationary free per slab must be 64 or 128)
    vp8 = persist.tile([P, SCH // 2, 2, HL, P], F8)
    oh = persist.tile([DH, HL, S], F16)

    a2a_in = [dram.tile([QBP, JC], F16, name=f"a2a_in{i}") for i in range(2)]
    a2a_out = [dram.tile([QBP, JC], F16, name=f"a2a_out{i}") for i in range(2)]

    # weight slices (fp8 for K-side DoubleRow, fp16 for Q)
    w_cm = tc.tile_pool(name="w", bufs=1)
    wpool = w_cm.__enter__()
    wk8 = wpool.tile([P, DCH, JC], F8)
    nc.gpsimd.dma_start(wk8[:], t["wk"].rearrange("(c p) j -> p c j", p=P))
    wkv8 = wpool.tile([P, DCH, JC], F8)
    nc.gpsimd.dma_start(wkv8[:], t["wkv"].rearrange("(c p) j -> p c j", p=P))
    wq_sb = wpool.tile([P, DCH, JC], F16)
    nc.gpsimd.dma_start(wq_sb[:], t["wq"].rearrange("(c p) j -> p c j", p=P))

    # ---- K path: transposes -> KpT (fp8 DoubleRow) -> k8; Vp -> vp8 ----
    xt_cm = tc.tile_pool(name="xt", bufs=1)
    xtp = xt_cm.__enter__()
    xt8 = xtp.tile([P, DCH, S], F8)
    xraw_cm = tc.tile_pool(name="xraw", bufs=3)
    pools["xraw"] = xraw_cm.__enter__()
    _transpose_in(nc, pools, t["k"], xt8, F8)
    xt8v = xt8.rearrange("p (g two) s -> p g two s", two=2)
    wk8v = wk8.rearrange("p (g two) j -> p g two j", two=2)
    wkv8v = wkv8.rearrange("p (g two) j -> p g two j", two=2)
    for jc2 in range(2):
        for sh in range(2):
            ps = pools["ps2"].tile([P, QBP], F32, tag="ps2")
            for dcp in range(4):
                for nb in range(2):
                    off = sh * QBP + nb * 512
                    nc.tensor.matmul(
                        ps[:, nb * 512:(nb + 1) * 512],
                        wk8v[:, dcp, :, jc2 * P:(jc2 + 1) * P],
                        xt8v[:, dcp, :, off:off + 512],
                        start=(dcp == 0), stop=(dcp == 3), perf_mode=DR)
            ssl = slice(sh * QBP, (sh + 1) * QBP)
            for hh in range(2):
                rsl = slice(hh * DH, (hh + 1) * DH)
                dst = k_heads[:, 2 * jc2 + hh, ssl]
                if hh == 0:
                    nc.scalar.activation(
                        dst, ps[rsl, :], AF.Identity,
                        bias=bkp[rsl, jc2:jc2 + 1], scale=1.0)
                else:
                    nc.vector.tensor_scalar(
                        out=dst, in0=ps[rsl, :],
                        scalar1=bkp[rsl, jc2:jc2 + 1], scalar2=None, op0=OP.add)
    # Vp natural [S, JC] = K @ (Wk Wv) via fp8 DoubleRow, + ones column
    for sc in range(SCH):
        psv = pools["ps2"].tile([P, QBP], F32, tag="ps2")
        for dcp in range(4):
            nc.tensor.matmul(
                psv[:, 0:JC], xt8v[:, dcp, :, sc * P:(sc + 1) * P],
                wkv8v[:, dcp, :, :], start=(dcp == 0), stop=(dcp == 3),
                perf_mode=DR)
        nc.vector.tensor_tensor(
            out=vp8[:, sc // 2, sc % 2, :, 0:DH],
            in0=psv[:, 0:JC].rearrange("p (h d) -> p h d", h=HL),
            in1=bvb.rearrange("p (h d) -> p h d", h=HL), op=OP.add)
    nc.gpsimd.memset(vp8[:, :, :, :, DH + 1:], 0.0)
    nc.gpsimd.memset(vp8[:, :, :, :, DH:DH + 1], 1.0)
    xraw_cm.__exit__(None, None, None)
    xt_cm.__exit__(None, None, None)

    # ---- attention/tail pools open before the Q-path xt pools so the
    # xt pools can close (LIFO) mid-attention and release their 38KB ----
    att_cm = tc.tile_pool(name="att", bufs=2)
    att = att_cm.__enter__()
    epool_cm = tc.tile_pool(name="epool", bufs=3)
    epool = epool_cm.__enter__()
    tail_cm = tc.tile_pool(name="tail", bufs=1)
    tailp = tail_cm.__enter__()

    # ---- Q path: transposes (fp16) ----
    xt_cm2 = tc.tile_pool(name="xt2", bufs=1)
    xtp2 = xt_cm2.__enter__()
    xtq = xtp2.tile([P, DCH, S], F16)
    xraw_cm2 = tc.tile_pool(name="xraw2", bufs=3)
    pools["xraw"] = xraw_cm2.__enter__()
    _transpose_in(nc, pools, t["q"], xtq, F16)

    def q_proj(jc2):
        """QpT for head pair jc2 -> q_heads (f16) + q8 (fp8)."""
        for sh in range(2):
            ps = pools["ps2"].tile([P, QBP], F32, tag="ps2")
            for dc in range(DCH):
                for nb in range(2):
                    off = sh * QBP + nb * 512
                    nc.tensor.matmul(
                        ps[:, nb * 512:(nb + 1) * 512],
                        wq_sb[:, dc, jc2 * P:(jc2 + 1) * P],
                        xtq[:, dc, off:off + 512],
                        start=(dc == 0), stop=(dc == DCH - 1))
            ssl = slice(sh * QBP, (sh + 1) * QBP)
            for hh in range(2):
                h = 2 * jc2 + hh
                rsl = slice(hh * DH, (hh + 1) * DH)
                if hh == 0:
                    nc.scalar.activation(
                        q_heads[:, h, ssl], ps[rsl, :], AF.Identity,
                        bias=bqp[rsl, jc2:jc2 + 1], scale=1.0)
                else:
                    nc.vector.tensor_scalar(
                        out=q_heads[:, h, ssl], in0=ps[rsl, :],
                        scalar1=bqp[rsl, jc2:jc2 + 1], scalar2=None, op0=OP.add)

    # ---- attention + chunked AllToAll + tail ----

    def att_unit(h, qbp):
        qsl = slice(qbp * QBP, (qbp + 1) * QBP)
        psA = ps_a.tile([P, QBP], F32, tag="psA")
        for kcp in range(SCH // 2):
            e2 = epool.tile([P, 2, 2, 512], F8E5, tag="e")  # [k, slab, nb, q]
            for i in range(2):
                kc = 2 * kcp + i
                pss = pools["ps2"].tile([P, QBP], F32, tag="ps2")
                for nb in range(2):
                    nc.tensor.matmul(
                        pss[:, nb * 512:(nb + 1) * 512],
                        k_heads[:, h, kc * P:(kc + 1) * P],
                        q_heads[:, h, qbp * QBP + nb * 512:qbp * QBP + (nb + 1) * 512],
                        start=True, stop=True)
                # bias -3: keeps exp within fp8e4m3 range (max 240); the
                # e^-3 factor cancels between numerator and denominator
                nc.scalar.activation(
                    e2[:, i, :, :].rearrange("p a b -> p (a b)"), pss[:],
                    AF.Exp, scale=0.125, bias=pools["neg3"][:])
            for nb in range(2):
                nc.tensor.matmul(
                    psA[:, nb * 512:(nb + 1) * 512],
                    vp8[:, kcp, :, h, :], e2[:, :, nb, :],
                    start=(kcp == 0), stop=(kcp == SCH // 2 - 1), perf_mode=DR)
        recip = att.tile([1, QBP], F32, tag="recip")
        nc.vector.reciprocal(recip[:], psA[DH:DH + 1, :])
        recipb = att.tile([DH, QBP], F32, tag="recipb")
        nc.gpsimd.partition_broadcast(recipb[:], recip[:], channels=DH)
        nc.vector.tensor_tensor(out=oh[:, h, qsl], in0=psA[0:DH, :],
                                in1=recipb[:], op=OP.mult)
        nc.vector.tensor_tensor(out=oh[:, h, qsl], in0=oh[:, h, qsl],
                                in1=q_heads[:, h, qsl], op=OP.add)

    def a2a_chunk(qbp):
        for scq in range(QBP // P):
            pstt = pools["pst"].tile([P, 4 * P], F16, tag="pst")
            for h in range(HL):
                nc.tensor.transpose(
                    pstt[:, h * DH:(h + 1) * DH],
                    oh[:, h, qbp * QBP + scq * P:qbp * QBP + (scq + 1) * P],
                    ident[0:DH, 0:DH])
            stg = att.tile([P, JC], F16, tag="stg")
            nc.vector.tensor_copy(stg[:], pstt[:, 0:JC])
            nc.sync.dma_start(a2a_in[qbp][scq * P:(scq + 1) * P, :], stg[:])
        nc.gpsimd.collective_compute(
            "AllToAll", OP.bypass, ins=[a2a_in[qbp].opt()],
            outs=[a2a_out[qbp].opt()], replica_groups=[list(range(NCORES))])

    def layernorm(src_ap, dst_ap, gb, bb):
        red = tailp.tile([P, 1], F32, tag="red")
        nc.vector.tensor_reduce(red[:], src_ap, mybir.AxisListType.X, OP.add)
        negmean = tailp.tile([P, 1], F32, tag="negmean")
        nc.vector.tensor_scalar_mul(negmean[:], red[:], -1.0 / D)
        sq = tailp.tile([P, D], F32, tag="scratchA")
        sumsq = tailp.tile([P, 1], F32, tag="sumsq")
        nc.scalar.activation(sq[:], src_ap, AF.Square, bias=negmean[:],
                             scale=1.0, accum_out=sumsq[:])
        std = tailp.tile([P, 1], F32, tag="std")
        nc.scalar.activation(std[:], sumsq[:], AF.Sqrt, bias=eps_t[:],
                             scale=1.0 / D)
        rstd = tailp.tile([P, 1], F32, tag="rstd")
        nc.vector.reciprocal(rstd[:], std[:])
        nc.vector.tensor_scalar(out=dst_ap, in0=src_ap, scalar1=negmean[:],
                                scalar2=rstd[:], op0=OP.add, op1=OP.mult)
        nc.gpsimd.tensor_tensor(out=dst_ap, in0=dst_ap, in1=gb[:], op=OP.mult)
        nc.gpsimd.tensor_tensor(out=dst_ap, in0=dst_ap, in1=bb[:], op=OP.add)

    def tail_block(qbp, b2):
        osb = tailp.tile([P, D], F16, tag="osb")
        for j in range(GROUP):
            nc.sync.dma_start(
                osb[:, j * JC:(j + 1) * JC],
                a2a_out[qbp][(GROUP * b2 + j) * P:(GROUP * b2 + j + 1) * P, :])
        ln0 = tailp.tile([P, D], F16, tag="ln0")
        layernorm(osb[:], ln0[:], g0b, b0b)
        ln0t = tailp.tile([P, DCH, P], F16, tag="ln0t")
        for dcg in range(2):
            pstt = pools["pst"].tile([P, 4 * P], F16, tag="pst")
            for i in range(4):
                dc = 4 * dcg + i
                nc.tensor.transpose(pstt[:, i * P:(i + 1) * P],
                                    ln0[:, dc * P:(dc + 1) * P], ident)
            nc.vector.tensor_copy(
                ln0t[:, 4 * dcg:4 * dcg + 4, :],
                pstt.rearrange("p (c q) -> p c q", c=4))
        pso = pools["ps2"].tile([P, QBP], F32, tag="ps2")
        for dc in range(DCH):
            for nb in range(2):
                nc.tensor.matmul(
                    pso[:, nb * 512:(nb + 1) * 512], ln0t[:, dc, :],
                    wo_sb[:, dc, nb * 512:(nb + 1) * 512],
                    start=(dc == 0), stop=(dc == DCH - 1))
        fb = tailp.tile([P, D], F32, tag="scratchA")
        nc.vector.tensor_tensor(out=fb[:], in0=pso[:], in1=bob[:], op=OP.add)
        gel = tailp.tile([P, D], F32, tag="gel")
        nc.scalar.activation(gel[:], fb[:], AF.Gelu)
        o2 = tailp.tile([P, D], F32, tag="o2")
        nc.vector.tensor_tensor(out=o2[:], in0=ln0[:], in1=gel[:], op=OP.add)
        fin = tailp.tile([P, D], F32, tag="fin")
        layernorm(o2[:], fin[:], g1b, b1b)
        nc.sync.dma_start(
            t["out"][(2 * qbp + b2) * P:(2 * qbp + b2 + 1) * P, :], fin[:])

    # interleaved emission: overlap Q projections with early attention,
    # and chunk-0 tail with the second half of attention
    q_proj(0)
    att_unit(0, 0)
    att_unit(1, 0)
    q_proj(1)
    att_unit(2, 0)
    att_unit(3, 0)
    xraw_cm2.__exit__(None, None, None)
    xt_cm2.__exit__(None, None, None)
    late_cm = tc.tile_pool(name="late", bufs=1)
    latep = late_cm.__enter__()
    wo_sb = latep.tile([P, DCH, D], F16)
    nc.gpsimd.dma_start(wo_sb[:], t["wo"].rearrange("(c p) j -> p c j", p=P))
    a2a_chunk(0)
    att_unit(0, 1)
    att_unit(1, 1)
    tail_block(0, 0)
    att_unit(2, 1)
    tail_block(0, 1)
    att_unit(3, 1)
    a2a_chunk(1)
    tail_block(1, 0)
    tail_block(1, 1)

    late_cm.__exit__(None, None, None)
    xraw_cm2 = None
    tail_cm.__exit__(None, None, None)
    epool_cm.__exit__(None, None, None)
    att_cm.__exit__(None, None, None)
    w_cm.__exit__(None, None, None)
    persist_cm.__exit__(None, None, None)


def build():
    if "nc" in _CACHE:
        return _CACHE["nc"]
    from contextlib import ExitStack
    nc = bacc.Bacc("TRN2", target_bir_lowering=False, debug=False,
                   num_devices=NCORES)
    t = _declare_io(nc)
    with tile.TileContext(nc) as tc:
        with ExitStack() as ctx:
            _emit(nc, tc, ctx, t)
    nc.compile()
    _CACHE["nc"] = nc
    return nc


def make_in_maps(Q, K, Wq, bq, Wk, bk, Wv, bv, Wo, bo, g0, b0, g1, b1):
    import ml_dtypes
    f16 = np.float16
    f32 = np.float32
    f8 = ml_dtypes.float8_e4m3
    Wkv = (Wk.astype(f32) @ Wv.astype(f32))
    bkv = (bk.astype(f32) @ Wv.astype(f32) + bv.astype(f32))
    Qh = [np.ascontiguousarray(Q[b].astype(f16)) for b in range(2)]
    Kh = [np.ascontiguousarray(K[b].astype(f16)) for b in range(2)]
    Wo16 = np.ascontiguousarray(Wo.astype(f16))
    in_maps = []
    for c in range(NCORES):
        b, g = divmod(c, GROUP)
        jsl = slice(g * JC, (g + 1) * JC)
        ac = np.ascontiguousarray
        in_maps.append({
            "q": Qh[b], "k": Kh[b],
            "wq": ac(Wq[:, jsl].astype(f16)),
            "wk": ac(Wk[:, jsl].astype(f8)),
            "wkv": ac(Wkv[:, jsl].astype(f8)),
            "bqp": ac(bq[jsl].astype(f32).reshape(2, P).T),
            "bkp": ac(bk[jsl].astype(f32).reshape(2, P).T),
            "bvv": ac(bkv[jsl].reshape(1, JC)),
            "wo": Wo16, "bo": ac(bo.astype(f32).reshape(1, D)),
            "g0": ac(g0.astype(f32).reshape(1, D)),
            "b0": ac(b0.astype(f32).reshape(1, D)),
            "g1": ac(g1.astype(f32).reshape(1, D)),
            "b1": ac(b1.astype(f32).reshape(1, D)),
        })
    return in_maps


def run(in_maps, trace=False, **kwargs):
    nc = build()
    return bass_utils.run_bass_kernel_spmd(
        nc, in_maps, core_ids=list(range(NCORES)), trace=trace, **kwargs)


def kernel(**inputs):
    inputs = {k: np.asarray(v) for k, v in inputs.items()}
    in_maps = make_in_maps(
        inputs["Q"], inputs["K"], inputs["Wq"], inputs["bq"], inputs["Wk"],
        inputs["bk"], inputs["Wv"], inputs["bv"], inputs["Wo"], inputs["bo"],
        inputs["g0"], inputs["b0"], inputs["g1"], inputs["b1"])
    res = run(in_maps, trace=False)
    out = np.empty((2, S, D), dtype=np.float32)
    for c in range(NCORES):
        r = res.results[c]["out"]  # [512, D] blocks: (qbp0,b0),(qbp0,b1),(qbp1,b0),(qbp1,b1)
        for qbp in range(2):
            for b in range(2):
                out[b, qbp * QBP + c * P:qbp * QBP + (c + 1) * P, :] = \
                    r[(2 * qbp + b) * P:(2 * qbp + b + 1) * P]
    return out


if __name__ == "__main__":
    rng = np.random.default_rng(0)
    ins = {n: rng.standard_normal(s).astype(np.float32) * (0.03125 if n.startswith("W") else 1.0)
           for n, s in [("Q", (2, S, D)), ("K", (2, S, D)), ("Wq", (D, D)),
                        ("Wk", (D, D)), ("Wv", (D, D)), ("Wo", (D, D))]}
    for n in ("bq", "bk", "bv", "bo", "b0", "b1"):
        ins[n] = np.zeros(D, np.float32)
    for n in ("g0", "g1"):
        ins[n] = np.ones(D, np.float32)
    out = kernel(**ins)
    print("ran ok", out.shape, out.dtype)


# revision 17
# speedup vs baseline: 1.6248x; 1.0148x over previous
"""Fused transformer block (QKV proj + attention + FFN + 2x LayerNorm) on 8
Trainium2 NeuronCores.

Sharding: batch (B=2) across two 4-core groups; within a group, tensor
parallel over heads (4 heads / core) for projections+attention, then a
2-chunk AllToAll switches to row sharding for the FFN/LayerNorm tail.

v3: fp8e4m3 DoubleRow matmuls (0.5 cyc/row) for scores, attnV, and the
K-side projections (K/V feed softmax paths where quantization noise
averages out; the 1/sqrt(dh) scale keeps score errors ~0.5%). Q path and
FFN stay fp16 for the residual/output precision. Host-folded Wkv = Wk@Wv
kills the AllGather; a 2-chunk fp16 AllToAll overlaps the tail with the
second half of attention; softmax exp owns the Act engine.
"""
import sys

import numpy as np

try:
    import concourse.bass  # noqa: F401
except ImportError:
    sys.path.insert(0, "/opt/trn_rl_repo")

import concourse.bacc as bacc
import concourse.mybir as mybir
import concourse.tile as tile
from concourse import bass_utils
from concourse.masks import make_identity

P = 128
S = 2048          # sequence length (Sq == Sk)
D = 1024          # model dim
H = 16            # total heads
DH = 64           # head dim
NCORES = 8
GROUP = 4         # cores per batch group
JC = D // GROUP   # 256 local projection columns
HL = JC // DH     # 4 local heads
DCH = D // P      # 8 d chunks
SCH = S // P      # 16 s chunks
QBP = 1024        # q rows per attention unit (2 units per head)
F32 = mybir.dt.float32
F16 = mybir.dt.float16
BF16 = mybir.dt.bfloat16
F8 = mybir.dt.float8e4
F8E5 = mybir.dt.float8e5
AF = mybir.ActivationFunctionType
OP = mybir.AluOpType
DR = mybir.MatmulPerfMode.DoubleRow
EPS = 1e-5

_CACHE: dict = {}


def _declare_io(nc):
    t = {}
    t["q"] = nc.dram_tensor("q", [S, D], F16, kind="ExternalInput").ap()
    t["k"] = nc.dram_tensor("k", [S, D], F16, kind="ExternalInput").ap()
    t["wq"] = nc.dram_tensor("wq", [D, JC], F16, kind="ExternalInput").ap()
    t["wk"] = nc.dram_tensor("wk", [D, JC], F8, kind="ExternalInput").ap()
    t["wkv"] = nc.dram_tensor("wkv", [D, JC], F8, kind="ExternalInput").ap()
    for b in ("bqp", "bkp"):
        t[b] = nc.dram_tensor(b, [P, 2], F32, kind="ExternalInput").ap()
    t["bvv"] = nc.dram_tensor("bvv", [1, JC], F32, kind="ExternalInput").ap()
    t["wo"] = nc.dram_tensor("wo", [D, D], F16, kind="ExternalInput").ap()
    for b in ("bo", "g0", "b0", "g1", "b1"):
        t[b] = nc.dram_tensor(b, [1, D], F32, kind="ExternalInput").ap()
    t["out"] = nc.dram_tensor("out", [4 * P, D], F32, kind="ExternalOutput").ap()
    return t


def _transpose_in(nc, pools, x_dram, xt, out_dt):
    """x [S, D] f16 DRAM -> xt [128, DCH, S] (dtype out_dt) via PE transposes."""
    ident = pools["ident"]
    for sc in range(SCH):
        xr = pools["xraw"].tile([P, D], F16, tag="xr")
        nc.sync.dma_start(xr[:], x_dram[sc * P:(sc + 1) * P, :])
        for dcg in range(2):
            pstt = pools["pst"].tile([P, 4 * P], F16, tag="pst")
            for i in range(4):
                dc = 4 * dcg + i
                nc.tensor.transpose(pstt[:, i * P:(i + 1) * P],
                                    xr[:, dc * P:(dc + 1) * P], ident)
            dst = xt[:, 4 * dcg:4 * dcg + 4, sc * P:(sc + 1) * P]
            src = pstt.rearrange("p (c q) -> p c q", c=4)
            if (sc + dcg) % 2 == 0:
                nc.vector.tensor_copy(dst, src)
            else:
                nc.scalar.copy(dst, src)


def _emit(nc, tc, ctx, t):
    pools = {}
    pools["const"] = ctx.enter_context(tc.tile_pool(name="const", bufs=1))
    persist_cm = tc.tile_pool(name="persist", bufs=1)
    persist = persist_cm.__enter__()
    dram = ctx.enter_context(tc.tile_pool(name="dram", bufs=1, space="DRAM"))
    pools["pst"] = ctx.enter_context(tc.tile_pool(name="pst", bufs=2, space="PSUM"))
    pools["ps2"] = ctx.enter_context(tc.tile_pool(name="ps2", bufs=2, space="PSUM"))
    ps_a = ctx.enter_context(tc.tile_pool(name="ps_a", bufs=1, space="PSUM"))
    const = pools["const"]

    # constants
    ident = const.tile([P, P], F16)
    make_identity(nc, ident)
    pools["ident"] = ident
    eps_t = const.tile([P, 1], F32)
    nc.vector.memset(eps_t, EPS)
    neg3 = const.tile([P, 1], F32)
    nc.vector.memset(neg3, -3.0)
    pools["neg3"] = neg3
    bqp = const.tile([P, 2], F32)
    nc.gpsimd.dma_start(bqp[:], t["bqp"])
    bkp = const.tile([P, 2], F32)
    nc.gpsimd.dma_start(bkp[:], t["bkp"])
    bvb = const.tile([P, JC], F32)
    nc.gpsimd.dma_start(bvb[:], t["bvv"].to_broadcast([P, JC]))
    bob = const.tile([P, D], F32)
    nc.gpsimd.dma_start(bob[:], t["bo"].to_broadcast([P, D]))
    g0b = const.tile([P, D], F32)
    nc.gpsimd.dma_start(g0b[:], t["g0"].to_broadcast([P, D]))
    b0b = const.tile([P, D], F32)
    nc.gpsimd.dma_start(b0b[:], t["b0"].to_broadcast([P, D]))
    g1b = const.tile([P, D], F32)
    nc.gpsimd.dma_start(g1b[:], t["g1"].to_broadcast([P, D]))
    b1b = const.tile([P, D], F32)
    nc.gpsimd.dma_start(b1b[:], t["b1"].to_broadcast([P, D]))

    # persistent tiles
    # k8/q8: [dh%32, dh//32 slab, head, s] fp8 for DoubleRow scores
    k8 = persist.tile([32, 2, HL, S], F8)
    q8 = persist.tile([32, 2, HL, S], F8)
    q_heads = persist.tile([DH, HL, S], F16)          # fp16 Qp^T for residual
    # vp8: [k%128, kc//2, kc%2 slab, head, dh+ones+zeros] fp8, M=128 for
    # DoubleRow attnV (stationary free per slab must be 64 or 128)
    vp8 = persist.tile([P, SCH // 2, 2, HL, P], F8)
    oh = persist.tile([DH, HL, S], F16)

    a2a_in = [dram.tile([QBP, JC], F16, name=f"a2a_in{i}") for i in range(2)]
    a2a_out = [dram.tile([QBP, JC], F16, name=f"a2a_out{i}") for i in range(2)]

    # weight slices (fp8 for K-side DoubleRow, fp16 for Q)
    w_cm = tc.tile_pool(name="w", bufs=1)
    wpool = w_cm.__enter__()
    wk8 = wpool.tile([P, DCH, JC], F8)
    nc.gpsimd.dma_start(wk8[:], t["wk"].rearrange("(c p) j -> p c j", p=P))
    wkv8 = wpool.tile([P, DCH, JC], F8)
    nc.gpsimd.dma_start(wkv8[:], t["wkv"].rearrange("(c p) j -> p c j", p=P))
    wq_sb = wpool.tile([P, DCH, JC], F16)
    nc.gpsimd.dma_start(wq_sb[:], t["wq"].rearrange("(c p) j -> p c j", p=P))

    # ---- K path: transposes -> KpT (fp8 DoubleRow) -> k8; Vp -> vp8 ----
    xt_cm = tc.tile_pool(name="xt", bufs=1)
    xtp = xt_cm.__enter__()
    xt8 = xtp.tile([P, DCH, S], F8)
    xraw_cm = tc.tile_pool(name="xraw", bufs=3)
    pools["xraw"] = xraw_cm.__enter__()
    _transpose_in(nc, pools, t["k"], xt8, F8)
    xt8v = xt8.rearrange("p (g two) s -> p g two s", two=2)
    wk8v = wk8.rearrange("p (g two) j -> p g two j", two=2)
    wkv8v = wkv8.rearrange("p (g two) j -> p g two j", two=2)
    for jc2 in range(2):
        for sh in range(2):
            ps = pools["ps2"].tile([P, QBP], F32, tag="ps2")
            for dcp in range(4):
                for nb in range(2):
                    off = sh * QBP + nb * 512
                    nc.tensor.matmul(
                        ps[:, nb * 512:(nb + 1) * 512],
                        wk8v[:, dcp, :, jc2 * P:(jc2 + 1) * P],
                        xt8v[:, dcp, :, off:off + 512],
                        start=(dcp == 0), stop=(dcp == 3), perf_mode=DR)
            ssl = slice(sh * QBP, (sh + 1) * QBP)
            for hh in range(2):
                for u in range(2):
                    rsl = slice(hh * DH + u * 32, hh * DH + u * 32 + 32)
                    dst = k8[:, u, 2 * jc2 + hh, ssl]
                    if u == 0:
                        nc.scalar.activation(
                            dst, ps[rsl, :], AF.Identity,
                            bias=bkp[rsl, jc2:jc2 + 1], scale=1.0)
                    else:
                        nc.vector.tensor_scalar(
                            out=dst, in0=ps[rsl, :],
                            scalar1=bkp[rsl, jc2:jc2 + 1], scalar2=None,
                            op0=OP.add)
    # Vp natural [S, JC] = K @ (Wk Wv) via fp8 DoubleRow, + ones column
    nc.gpsimd.memset(vp8[:, :, :, :, DH + 1:], 0.0)
    nc.gpsimd.memset(vp8[:, :, :, :, DH:DH + 1], 1.0)
    for sc in range(SCH):
        psv = pools["ps2"].tile([P, QBP], F32, tag="ps2")
        for dcp in range(4):
            nc.tensor.matmul(
                psv[:, 0:JC], xt8v[:, dcp, :, sc * P:(sc + 1) * P],
                wkv8v[:, dcp, :, :], start=(dcp == 0), stop=(dcp == 3),
                perf_mode=DR)
        nc.vector.tensor_tensor(
            out=vp8[:, sc // 2, sc % 2, :, 0:DH],
            in0=psv[:, 0:JC].rearrange("p (h d) -> p h d", h=HL),
            in1=bvb.rearrange("p (h d) -> p h d", h=HL), op=OP.add)
    xraw_cm.__exit__(None, None, None)
    xt_cm.__exit__(None, None, None)

    # ---- attention/tail pools open before the Q-path xt pools so the
    # xt pools can close (LIFO) mid-attention and release their 38KB ----
    att_cm = tc.tile_pool(name="att", bufs=2)
    att = att_cm.__enter__()
    epool_cm = tc.tile_pool(name="epool", bufs=3)
    epool = epool_cm.__enter__()
    tail_cm = tc.tile_pool(name="tail", bufs=1)
    tailp = tail_cm.__enter__()

    # ---- Q path: transposes (fp16) ----
    xt_cm2 = tc.tile_pool(name="xt2", bufs=1)
    xtp2 = xt_cm2.__enter__()
    xtq = xtp2.tile([P, DCH, S], F16)
    xraw_cm2 = tc.tile_pool(name="xraw2", bufs=3)
    pools["xraw"] = xraw_cm2.__enter__()
    _transpose_in(nc, pools, t["q"], xtq, F16)

    def q_proj(jc2):
        """QpT for head pair jc2 -> q_heads (f16) + q8 (fp8)."""
        for sh in range(2):
            ps = pools["ps2"].tile([P, QBP], F32, tag="ps2")
            for dc in range(DCH):
                for nb in range(2):
                    off = sh * QBP + nb * 512
                    nc.tensor.matmul(
                        ps[:, nb * 512:(nb + 1) * 512],
                        wq_sb[:, dc, jc2 * P:(jc2 + 1) * P],
                        xtq[:, dc, off:off + 512],
                        start=(dc == 0), stop=(dc == DCH - 1))
            ssl = slice(sh * QBP, (sh + 1) * QBP)
            for hh in range(2):
                h = 2 * jc2 + hh
                rsl = slice(hh * DH, (hh + 1) * DH)
                if hh == 0:
                    nc.scalar.activation(
                        q_heads[:, h, ssl], ps[rsl, :], AF.Identity,
                        bias=bqp[rsl, jc2:jc2 + 1], scale=1.0)
                else:
                    nc.vector.tensor_scalar(
                        out=q_heads[:, h, ssl], in0=ps[rsl, :],
                        scalar1=bqp[rsl, jc2:jc2 + 1], scalar2=None, op0=OP.add)
                for u in range(2):
                    r2 = slice(hh * DH + u * 32, hh * DH + u * 32 + 32)
                    dst = q8[:, u, h, ssl]
                    if u == hh:
                        nc.scalar.activation(
                            dst, ps[r2, :], AF.Identity,
                            bias=bqp[r2, jc2:jc2 + 1], scale=1.0)
                    else:
                        nc.vector.tensor_scalar(
                            out=dst, in0=ps[r2, :],
                            scalar1=bqp[r2, jc2:jc2 + 1], scalar2=None,
                            op0=OP.add)

    # ---- attention + chunked AllToAll + tail ----

    def att_unit(h, qbp):
        qsl = slice(qbp * QBP, (qbp + 1) * QBP)
        psA = ps_a.tile([P, QBP], F32, tag="psA")
        for kcp in range(SCH // 2):
            e2 = epool.tile([P, 2, 2, 512], F8E5, tag="e")  # [k, slab, nb, q]
            for i in range(2):
                kc = 2 * kcp + i
                pss = pools["ps2"].tile([P, QBP], F32, tag="ps2")
                for nb in range(2):
                    nc.tensor.matmul(
                        pss[:, nb * 512:(nb + 1) * 512],
                        k8[:, :, h, kc * P:(kc + 1) * P],
                        q8[:, :, h, qbp * QBP + nb * 512:qbp * QBP + (nb + 1) * 512],
                        start=True, stop=True, perf_mode=DR)
                # bias -3: keeps exp within fp8e4m3 range (max 240); the
                # e^-3 factor cancels between numerator and denominator
                nc.scalar.activation(
                    e2[:, i, :, :].rearrange("p a b -> p (a b)"), pss[:],
                    AF.Exp, scale=0.125, bias=pools["neg3"][:])
            for nb in range(2):
                nc.tensor.matmul(
                    psA[:, nb * 512:(nb + 1) * 512],
                    vp8[:, kcp, :, h, :], e2[:, :, nb, :],
                    start=(kcp == 0), stop=(kcp == SCH // 2 - 1), perf_mode=DR)
        recip = att.tile([1, QBP], F32, tag="recip")
        nc.vector.reciprocal(recip[:], psA[DH:DH + 1, :])
        recipb = att.tile([DH, QBP], F32, tag="recipb")
        nc.gpsimd.partition_broadcast(recipb[:], recip[:], channels=DH)
        nc.vector.tensor_tensor(out=oh[:, h, qsl], in0=psA[0:DH, :],
                                in1=recipb[:], op=OP.mult)
        nc.vector.tensor_tensor(out=oh[:, h, qsl], in0=oh[:, h, qsl],
                                in1=q_heads[:, h, qsl], op=OP.add)

    def a2a_chunk(qbp):
        for scq in range(QBP // P):
            pstt = pools["pst"].tile([P, 4 * P], F16, tag="pst")
            for h in range(HL):
                nc.tensor.transpose(
                    pstt[:, h * DH:(h + 1) * DH],
                    oh[:, h, qbp * QBP + scq * P:qbp * QBP + (scq + 1) * P],
                    ident[0:DH, 0:DH])
            stg = att.tile([P, JC], F16, tag="stg")
            nc.vector.tensor_copy(stg[:], pstt[:, 0:JC])
            nc.sync.dma_start(a2a_in[qbp][scq * P:(scq + 1) * P, :], stg[:])
        nc.gpsimd.collective_compute(
            "AllToAll", OP.bypass, ins=[a2a_in[qbp].opt()],
            outs=[a2a_out[qbp].opt()], replica_groups=[list(range(NCORES))])

    def layernorm(tp, sfx, src_ap, dst_ap, gb, bb):
        red = tp.tile([P, 1], F32, tag="red" + sfx)
        nc.vector.tensor_reduce(red[:], src_ap, mybir.AxisListType.X, OP.add)
        negmean = tp.tile([P, 1], F32, tag="negmean" + sfx)
        nc.vector.tensor_scalar_mul(negmean[:], red[:], -1.0 / D)
        sq = tp.tile([P, D], F32, tag="scratchA" + sfx)
        sumsq = tp.tile([P, 1], F32, tag="sumsq" + sfx)
        nc.scalar.activation(sq[:], src_ap, AF.Square, bias=negmean[:],
                             scale=1.0, accum_out=sumsq[:])
        std = tp.tile([P, 1], F32, tag="std" + sfx)
        nc.scalar.activation(std[:], sumsq[:], AF.Sqrt, bias=eps_t[:],
                             scale=1.0 / D)
        rstd = tp.tile([P, 1], F32, tag="rstd" + sfx)
        nc.vector.reciprocal(rstd[:], std[:])
        nc.vector.tensor_scalar(out=dst_ap, in0=src_ap, scalar1=negmean[:],
                                scalar2=rstd[:], op0=OP.add, op1=OP.mult)
        nc.gpsimd.tensor_tensor(out=dst_ap, in0=dst_ap, in1=gb[:], op=OP.mult)
        nc.gpsimd.tensor_tensor(out=dst_ap, in0=dst_ap, in1=bb[:], op=OP.add)

    def tail_block(qbp, b2, tp=None, sfx=""):
        tp = tp or tailp
        osb = tp.tile([P, D], F16, tag="osb" + sfx)
        for j in range(GROUP):
            nc.sync.dma_start(
                osb[:, j * JC:(j + 1) * JC],
                a2a_out[qbp][(GROUP * b2 + j) * P:(GROUP * b2 + j + 1) * P, :])
        ln0 = tp.tile([P, D], F16, tag="ln0" + sfx)
        layernorm(tp, sfx, osb[:], ln0[:], g0b, b0b)
        ln0t = tp.tile([P, DCH, P], F16, tag="ln0t" + sfx)
        for dcg in range(2):
            pstt = pools["pst"].tile([P, 4 * P], F16, tag="pst")
            for i in range(4):
                dc = 4 * dcg + i
                nc.tensor.transpose(pstt[:, i * P:(i + 1) * P],
                                    ln0[:, dc * P:(dc + 1) * P], ident)
            nc.vector.tensor_copy(
                ln0t[:, 4 * dcg:4 * dcg + 4, :],
                pstt.rearrange("p (c q) -> p c q", c=4))
        pso = pools["ps2"].tile([P, QBP], F32, tag="ps2")
        for dc in range(DCH):
            for nb in range(2):
                nc.tensor.matmul(
                    pso[:, nb * 512:(nb + 1) * 512], ln0t[:, dc, :],
                    wo_sb[:, dc, nb * 512:(nb + 1) * 512],
                    start=(dc == 0), stop=(dc == DCH - 1))
        fb = tp.tile([P, D], F32, tag="scratchA" + sfx)
        nc.vector.tensor_tensor(out=fb[:], in0=pso[:], in1=bob[:], op=OP.add)
        gel = tp.tile([P, D], F32, tag="gel" + sfx)
        nc.scalar.activation(gel[:], fb[:], AF.Gelu)
        o2 = tp.tile([P, D], F32, tag="o2" + sfx)
        nc.vector.tensor_tensor(out=o2[:], in0=ln0[:], in1=gel[:], op=OP.add)
        fin = tp.tile([P, D], F32, tag="fin" + sfx)
        layernorm(tp, sfx, o2[:], fin[:], g1b, b1b)
        nc.sync.dma_start(
            t["out"][(2 * qbp + b2) * P:(2 * qbp + b2 + 1) * P, :], fin[:])

    # interleaved emission: overlap Q projections with early attention,
    # and chunk-0 tail with the second half of attention
    q_proj(0)
    att_unit(0, 0)
    att_unit(1, 0)
    q_proj(1)
    att_unit(2, 0)
    att_unit(3, 0)
    xraw_cm2.__exit__(None, None, None)
    xt_cm2.__exit__(None, None, None)
    late_cm = tc.tile_pool(name="late", bufs=1)
    latep = late_cm.__enter__()
    wo_sb = latep.tile([P, DCH, D], F16)
    nc.gpsimd.dma_start(wo_sb[:], t["wo"].rearrange("(c p) j -> p c j", p=P))
    a2a_chunk(0)
    att_unit(0, 1)
    att_unit(1, 1)
    tail_block(0, 0)
    att_unit(2, 1)
    tail_block(0, 1)
    att_unit(3, 1)
    a2a_chunk(1)
    tail2_cm = tc.tile_pool(name="tail2", bufs=1)
    tail2p = tail2_cm.__enter__()
    tail_block(1, 0, tp=tail2p, sfx="a")
    tail_block(1, 1, tp=tail2p, sfx="b")

    tail2_cm.__exit__(None, None, None)
    late_cm.__exit__(None, None, None)
    xraw_cm2 = None
    tail_cm.__exit__(None, None, None)
    epool_cm.__exit__(None, None, None)
    att_cm.__exit__(None, None, None)
    w_cm.__exit__(None, None, None)
    persist_cm.__exit__(None, None, None)


def build():
    if "nc" in _CACHE:
        return _CACHE["nc"]
    from contextlib import ExitStack
    nc = bacc.Bacc("TRN2", target_bir_lowering=False, debug=False,
                   num_devices=NCORES)
    t = _declare_io(nc)
    with tile.TileContext(nc) as tc:
        with ExitStack() as ctx:
            _emit(nc, tc, ctx, t)
    nc.compile()
    _CACHE["nc"] = nc
    return nc


def make_in_maps(Q, K, Wq, bq, Wk, bk, Wv, bv, Wo, bo, g0, b0, g1, b1):
    import ml_dtypes
    f16 = np.float16
    f32 = np.float32
    f8 = ml_dtypes.float8_e4m3
    Wkv = (Wk.astype(f32) @ Wv.astype(f32))
    bkv = (bk.astype(f32) @ Wv.astype(f32) + bv.astype(f32))
    Qh = [np.ascontiguousarray(Q[b].astype(f16)) for b in range(2)]
    Kh = [np.ascontiguousarray(K[b].astype(f16)) for b in range(2)]
    Wo16 = np.ascontiguousarray(Wo.astype(f16))
    in_maps = []
    for c in range(NCORES):
        b, g = divmod(c, GROUP)
        jsl = slice(g * JC, (g + 1) * JC)
        ac = np.ascontiguousarray
        in_maps.append({
            "q": Qh[b], "k": Kh[b],
            "wq": ac(Wq[:, jsl].astype(f16)),
            "wk": ac(Wk[:, jsl].astype(f8)),
            "wkv": ac(Wkv[:, jsl].astype(f8)),
            "bqp": ac(bq[jsl].astype(f32).reshape(2, P).T),
            "bkp": ac(bk[jsl].astype(f32).reshape(2, P).T),
            "bvv": ac(bkv[jsl].reshape(1, JC)),
            "wo": Wo16, "bo": ac(bo.astype(f32).reshape(1, D)),
            "g0": ac(g0.astype(f32).reshape(1, D)),
            "b0": ac(b0.astype(f32).reshape(1, D)),
            "g1": ac(g1.astype(f32).reshape(1, D)),
            "b1": ac(b1.astype(f32).reshape(1, D)),
        })
    return in_maps


def run(in_maps, trace=False, **kwargs):
    nc = build()
    return bass_utils.run_bass_kernel_spmd(
        nc, in_maps, core_ids=list(range(NCORES)), trace=trace, **kwargs)


def kernel(**inputs):
    inputs = {k: np.asarray(v) for k, v in inputs.items()}
    in_maps = make_in_maps(
        inputs["Q"], inputs["K"], inputs["Wq"], inputs["bq"], inputs["Wk"],
        inputs["bk"], inputs["Wv"], inputs["bv"], inputs["Wo"], inputs["bo"],
        inputs["g0"], inputs["b0"], inputs["g1"], inputs["b1"])
    res = run(in_maps, trace=False)
    out = np.empty((2, S, D), dtype=np.float32)
    for c in range(NCORES):
        r = res.results[c]["out"]  # [512, D] blocks: (qbp0,b0),(qbp0,b1),(qbp1,b0),(qbp1,b1)
        for qbp in range(2):
            for b in range(2):
                out[b, qbp * QBP + c * P:qbp * QBP + (c + 1) * P, :] = \
                    r[(2 * qbp + b) * P:(2 * qbp + b + 1) * P]
    return out


if __name__ == "__main__":
    rng = np.random.default_rng(0)
    ins = {n: rng.standard_normal(s).astype(np.float32) * (0.03125 if n.startswith("W") else 1.0)
           for n, s in [("Q", (2, S, D)), ("K", (2, S, D)), ("Wq", (D, D)),
                        ("Wk", (D, D)), ("Wv", (D, D)), ("Wo", (D, D))]}
    for n in ("bq", "bk", "bv", "bo", "b0", "b1"):
        ins[n] = np.zeros(D, np.float32)
    for n in ("g0", "g1"):
        ins[n] = np.ones(D, np.float32)
    out = kernel(**ins)
    print("ran ok", out.shape, out.dtype)
